# revision 1
# baseline (speedup 1.0000x reference)
"""Trainium2 Bass kernel for nn_Attention_32762010534254.

Cross-attention: q(B,Nq,D) kv(B,Nkv,D) -> softmax((qWq)(kvWk)^T/sqrt(dh)) (kvWv) Wo + bo
B=2, Nq=512, Nkv=4096, D=1024, heads=16, dh=64.

Sharding (8 cores): core i handles batch b=i//4 and head group g=i%4
(4 heads = 2 pairs). Per-core device work:
  - K^T, V projections streamed over Nkv in 512-key chunks (f32r matmuls)
  - S^T = K_h Q_h^T with keys on partitions (row-tiled head pairs, K=64)
  - exp via ACT with fused 1/8 scale + per-key mask bias (no max subtraction:
    scores are O(1) by construction, fp32 exp cannot overflow)
  - O^T (+ softmax sums via a ones column appended to V) accumulated in PSUM
  - normalize via PE-broadcast reciprocal, row-parallel Wo partial product
Host: shards inputs (transposes q/kv once), sums the 4 partials per batch, +bo.

Self-contained: hardcodes all shapes; requires only concourse + numpy.
"""

import numpy as np

import concourse.bass as bass  # noqa: F401  (bass types via bacc/tile)
import concourse.tile as tile
from concourse import bacc, mybir
from concourse import bass_utils

F32 = mybir.dt.float32
F32R = mybir.dt.float32r
EXP = mybir.ActivationFunctionType.Exp

B, NQ, NKV, D = 2, 512, 4096, 1024
HEADS, DH = 16, 64
SCALE = DH ** -0.5
N_CORES = 8
HPC = HEADS // (N_CORES // B)   # heads per core = 4
PAIRS = HPC // 2                # head pairs per core = 2
KC = 512                        # Nkv chunk size
NCHUNK = NKV // KC              # 8
KO = D // 128                   # 8 contraction sub-chunks

import os
KV_BUFS = int(os.environ.get("KV_BUFS", "2"))
AV_LAG = int(os.environ.get("AV_LAG", "12"))
PT_BUFS = int(os.environ.get("PT_BUFS", "20"))
PSS_BUFS = int(os.environ.get("PSS_BUFS", "2"))
_NC_CACHE = []


def _build_nc():
    nc = bacc.Bacc("TRN2", target_bir_lowering=False, debug=False,
                   num_devices=N_CORES)
    qT = nc.dram_tensor("qT", [D, NQ], F32R, kind="ExternalInput").ap()
    kvT = nc.dram_tensor("kvT", [D, NKV], F32R, kind="ExternalInput").ap()
    wq = nc.dram_tensor("wq", [D, HPC * DH], F32R, kind="ExternalInput").ap()
    wkvk = nc.dram_tensor("wkvk", [D, HPC * DH], F32R, kind="ExternalInput").ap()
    wkvv = nc.dram_tensor("wkvv", [D, HPC * DH], F32R, kind="ExternalInput").ap()
    wo = nc.dram_tensor("wo", [HPC * DH, D], F32R, kind="ExternalInput").ap()
    bias = nc.dram_tensor("bias", [128, NCHUNK * 4], F32, kind="ExternalInput").ap()
    out = nc.dram_tensor("out", [NQ, D], F32, kind="ExternalOutput").ap()

    qT_r = qT.rearrange("(ko p) n -> p ko n", p=128)
    kvT_r = kvT.rearrange("(ko p) n -> p ko n", p=128)
    wq_r = wq.rearrange("(ko p) m -> p ko m", p=128)
    wkvk_r = wkvk.rearrange("(ko p) m -> p ko m", p=128)
    wkvv_r = wkvv.rearrange("(ko p) m -> p ko m", p=128)
    wo_r = wo.rearrange("(ko p) n -> p ko n", p=128)

    with tile.TileContext(nc) as tc:
        with (
            tc.tile_pool(name="const", bufs=1) as cpool,
            tc.tile_pool(name="kv", bufs=KV_BUFS) as kv_pool,
            tc.tile_pool(name="kt", bufs=2) as kt_pool,
            tc.tile_pool(name="v", bufs=2) as v_pool,
            tc.tile_pool(name="pt", bufs=PT_BUFS) as p_pool,
            tc.tile_pool(name="xo", bufs=1) as x_pool,
            tc.tile_pool(name="ob", bufs=8) as o_pool,
            tc.tile_pool(name="psA", bufs=1, space="PSUM") as psA,
            tc.tile_pool(name="psV", bufs=1, space="PSUM") as psV,
            tc.tile_pool(name="psS", bufs=PSS_BUFS, space="PSUM") as psS,
            tc.tile_pool(name="psO", bufs=1, space="PSUM") as psO,
        ):
            wq_sb = cpool.tile([128, KO, HPC * DH], F32R, tag="wq")
            wkvk_sb = cpool.tile([128, KO, HPC * DH], F32R, tag="wkvk")
            wkvv_sb = cpool.tile([128, KO, HPC * DH], F32R, tag="wkvv")
            wo_sb = cpool.tile([128, PAIRS, D], F32R, tag="wo")
            qT_sb = cpool.tile([128, KO, NQ], F32R, tag="qT")
            bias_sb = cpool.tile([128, NCHUNK * 4], F32, tag="bias")
            ones_sb = cpool.tile([1, DH], F32R, tag="ones")
            qh_sb = cpool.tile([128, PAIRS, NQ], F32R, tag="qh")

            nc.sync.dma_start(wkvk_sb[:], wkvk_r)
            # first kv chunk, split per-ko so K-proj can start on strip 0
            kvc0 = kv_pool.tile([128, KO, KC], F32R, tag="kvc", name="kvc0")
            for ko in range(KO):
                nc.sync.dma_start(kvc0[:, ko, :], kvT_r[:, ko, 0:KC])
            nc.sync.dma_start(wkvv_sb[:], wkvv_r)
            nc.sync.dma_start(wq_sb[:], wq_r)
            nc.sync.dma_start(qT_sb[:], qT_r)
            nc.sync.dma_start(bias_sb[:], bias)
            nc.vector.memset(ones_sb[:].bitcast(F32), 1.0)

            def q_projection():
                for p in range(PAIRS):
                    qp = psA.tile([128, NQ], F32, tag="psA", name="qp")
                    for ko in range(KO):
                        nc.tensor.matmul(
                            qp[:], wq_sb[:, ko, 128 * p:128 * (p + 1)],
                            qT_sb[:, ko, :], start=(ko == 0), stop=(ko == KO - 1),
                        )
                    nc.vector.tensor_copy(qh_sb[:, p, :], qp[:])

            # persistent O^T accumulators (one bank per head): rows 0:64 = O^T,
            # row 64 = softmax sums (ones column of V_aug)
            opsum = [psO.tile([65, NQ], F32, tag=f"o{h}", name=f"opsum{h}") for h in range(HPC)]

            def load_chunk(c):
                if c == 0:
                    return kvc0
                kvc = kv_pool.tile([128, KO, KC], F32R, tag="kvc", name=f"kvc{c}")
                nc.sync.dma_start(kvc[:], kvT_r[:, :, KC * c:KC * (c + 1)])
                return kvc

            def proj_k_pair(c, kvc, ktc, p):
                kp = psA.tile([128, KC], F32, tag="psA", name=f"kp{c}_{p}")
                for ko in range(KO):
                    nc.tensor.matmul(
                        kp[:], wkvk_sb[:, ko, 128 * p:128 * (p + 1)],
                        kvc[:, ko, :], start=(ko == 0), stop=(ko == KO - 1),
                    )
                nc.vector.tensor_copy(ktc[:, p, :], kp[:])

            def proj_v_sub(c, kvc, vc, m):
                vp = psV.tile([128, HPC * DH], F32, tag="psV", name=f"vp{c}_{m}")
                for ko in range(KO):
                    nc.tensor.matmul(
                        vp[:], kvc[:, ko, 128 * m:128 * (m + 1)],
                        wkvv_sb[:, ko, :], start=(ko == 0), stop=(ko == KO - 1),
                    )
                nc.vector.tensor_copy(
                    vc[:, m, :, 0:DH],
                    vp[:].rearrange("p (h d) -> p h d", h=HPC),
                )

            def alloc_proj_tiles(c):
                ktc = kt_pool.tile([128, PAIRS, KC], F32R, tag="ktc", name=f"ktc{c}")
                vc = v_pool.tile([128, 4, HPC, DH + 1], F32R, tag="vc", name=f"vc{c}")
                nc.vector.memset(vc[:, :, :, DH:DH + 1].bitcast(F32), 1.0)
                return ktc, vc

            def proj_piece(c, kvc, ktc, vc, s):
                # 4 pieces per chunk, each ~same PE duration as one attention s-group
                if s == 0:
                    proj_k_pair(c, kvc, ktc, 0)
                elif s == 1:
                    proj_k_pair(c, kvc, ktc, 1)
                elif s == 2:
                    proj_v_sub(c, kvc, vc, 0)
                    proj_v_sub(c, kvc, vc, 1)
                else:
                    proj_v_sub(c, kvc, vc, 2)
                    proj_v_sub(c, kvc, vc, 3)

            av_pending = []

            def qk_exp_group(c, ktc, vc, s):
                bias_ap = bias_sb[:, 4 * c + s:4 * c + s + 1]
                for p in range(PAIRS):
                    sps = []
                    for half in range(2):  # row-tiled pair, K=64
                        lo, hi = 64 * half, 64 * (half + 1)
                        sp = psS.tile([128, NQ], F32, tag="psS", name=f"sp{c}_{s}_{p}_{half}")
                        nc.tensor.matmul(
                            sp[:], ktc[lo:hi, p, 128 * s:128 * (s + 1)],
                            qh_sb[lo:hi, p, :], start=True, stop=True,
                        )
                        sps.append(sp)
                    for half, sp in enumerate(sps):
                        h = 2 * p + half
                        pt = p_pool.tile([128, NQ], F32R, tag="pt", name=f"pt{c}_{s}_{p}_{half}")
                        nc.scalar.activation(
                            pt[:], sp[:], EXP, bias=bias_ap, scale=SCALE,
                        )
                        av_pending.append((c, s, h, vc, pt))

            def flush_av(upto):
                # emit AV matmuls for groups that have lagged enough
                while av_pending and len(av_pending) > upto:
                    c, s, h, vc, pt = av_pending.pop(0)
                    nc.tensor.matmul(
                        opsum[h][:], vc[:, s, h, :], pt[:],
                        start=(c == 0 and s == 0),
                        stop=(c == NCHUNK - 1 and s == 3),
                        skip_group_check=True,
                    )

            # prologue: chunk 0 projections, then Q projection
            kvc_cur = load_chunk(0)
            ktc_cur, vc_cur = alloc_proj_tiles(0)
            for s in range(4):
                proj_piece(0, kvc_cur, ktc_cur, vc_cur, s)
            q_projection()

            # steady state: attention(c) interleaved with projections(c+1);
            # AV lags one s-group behind its exp to hide ACT->PE sem latency
            for c in range(NCHUNK):
                if c + 1 < NCHUNK:
                    kvc_nxt = load_chunk(c + 1)
                    ktc_nxt, vc_nxt = alloc_proj_tiles(c + 1)
                for s in range(4):
                    qk_exp_group(c, ktc_cur, vc_cur, s)
                    if os.environ.get("PROJ_FIRST") == "1":
                        if c + 1 < NCHUNK:
                            proj_piece(c + 1, kvc_nxt, ktc_nxt, vc_nxt, s)
                        flush_av(AV_LAG)
                    else:
                        flush_av(AV_LAG)
                        if c + 1 < NCHUNK:
                            proj_piece(c + 1, kvc_nxt, ktc_nxt, vc_nxt, s)
                if c + 1 < NCHUNK:
                    kvc_cur, ktc_cur, vc_cur = kvc_nxt, ktc_nxt, vc_nxt
            flush_av(0)

            # normalize: X^T[p] rows 64b:64b+64 = O^T_h * (1/sums_h) broadcast.
            # recip on DVE, partition-broadcast via K=1 PE matmul + ACT copy.
            xT = [x_pool.tile([128, NQ], F32R, tag=f"x{p}", name=f"xT{p}") for p in range(PAIRS)]
            rts, bps, bsbs = [], [], []
            for h in range(HPC):
                rt = cpool.tile([1, NQ], F32R, tag=f"r{h}", name=f"rt{h}")
                with nc.allow_low_precision(reason="softmax recip rounded to f32r"):
                    nc.vector.reciprocal(rt[:], opsum[h][64:65, :])
                rts.append(rt)
            for h in range(HPC):
                bp = psS.tile([64, NQ], F32, tag="psS", name=f"bp{h}")
                nc.tensor.matmul(bp[:], ones_sb[:], rts[h][:], start=True, stop=True)
                bps.append(bp)
                bsb = cpool.tile([64, NQ], F32, tag=f"b{h}", name=f"bsb{h}")
                nc.scalar.copy(bsb[:], bp[:])
                bsbs.append(bsb)
            for h in range(HPC):
                p, half = h // 2, h % 2
                nc.vector.tensor_mul(
                    xT[p][64 * half:64 * (half + 1), :], opsum[h][0:64, :], bsbs[h][:],
                )

            nc.sync.dma_start(wo_sb[:], wo_r)

            # Wo row-parallel partial: out = X^T.T @ Wo_slice
            wo_pools = [psS, psA, psV, psS, psA, psV, psS, psA]
            for mq in range(NQ // 128):
                for n in range(D // 512):
                    pool_w = wo_pools[mq * (D // 512) + n]
                    wp = pool_w.tile([128, 512], F32, tag=pool_w.name, name=f"wp{mq}_{n}")
                    for p in range(PAIRS):
                        nc.tensor.matmul(
                            wp[:], xT[p][:, 128 * mq:128 * (mq + 1)],
                            wo_sb[:, p, 512 * n:512 * (n + 1)],
                            start=(p == 0), stop=(p == PAIRS - 1),
                        )
                    osb = o_pool.tile([128, 512], F32, tag="osb")
                    if (mq * (D // 512) + n) % 2 == 0:
                        nc.vector.tensor_copy(osb[:], wp[:])
                    else:
                        nc.scalar.copy(osb[:], wp[:])
                    nc.sync.dma_start(
                        out[128 * mq:128 * (mq + 1), 512 * n:512 * (n + 1)], osb[:],
                    )

    nc.compile()
    return nc


def _get_nc():
    if not _NC_CACHE:
        _NC_CACHE.append(_build_nc())
    return _NC_CACHE[0]


LAST_RESULTS = None


def kernel(q, kv, mask, Wq, Wkv, Wo, bo):
    global LAST_RESULTS
    q = np.asarray(q, dtype=np.float32)
    kv = np.asarray(kv, dtype=np.float32)
    mask = np.asarray(mask)
    Wq = np.asarray(Wq, dtype=np.float32)
    Wkv = np.asarray(Wkv, dtype=np.float32)
    Wo = np.asarray(Wo, dtype=np.float32)
    bo = np.asarray(bo, dtype=np.float32)

    inner = HEADS * DH
    qT = [np.ascontiguousarray(q[b].T) for b in range(B)]
    kvT = [np.ascontiguousarray(kv[b].T) for b in range(B)]
    bias = []
    for b in range(B):
        bb = np.where(mask[b], 0.0, -30000.0).astype(np.float32)
        bias.append(np.ascontiguousarray(bb.reshape(NCHUNK * 4, 128).T))

    in_maps = []
    for i in range(N_CORES):
        b, g = divmod(i, N_CORES // B)
        cs = slice(HPC * DH * g, HPC * DH * (g + 1))
        in_maps.append({
            "qT": qT[b],
            "kvT": kvT[b],
            "wq": np.ascontiguousarray(Wq[:, cs]),
            "wkvk": np.ascontiguousarray(Wkv[:, cs]),
            "wkvv": np.ascontiguousarray(Wkv[:, inner:][:, cs]),
            "wo": np.ascontiguousarray(Wo[cs, :]),
            "bias": bias[b],
        })

    nc = _get_nc()
    res = bass_utils.run_bass_kernel_spmd(
        nc, in_maps, core_ids=list(range(N_CORES)))
    LAST_RESULTS = res

    gpb = N_CORES // B
    out = np.zeros((B, NQ, D), np.float32)
    for b in range(B):
        acc = res.results[b * gpb]["out"].astype(np.float32).copy()
        for g in range(1, gpb):
            acc += res.results[b * gpb + g]["out"]
        out[b] = acc + bo[None, :]
    return out



# revision 2
# speedup vs baseline: 1.0860x; 1.0860x over previous
"""Trainium2 Bass kernel for nn_Attention_32762010534254.

Cross-attention: q(B,Nq,D) kv(B,Nkv,D) -> softmax((qWq)(kvWk)^T/sqrt(dh)) (kvWv) Wo + bo
B=2, Nq=512, Nkv=4096, D=1024, heads=16, dh=64.

Sharding (8 cores): core i handles batch b=i//4 and head group g=i%4
(4 heads = 2 pairs). Per-core device work:
  - inputs DMA'd as bf16 (host-side cast); all matmuls bf16 or f32r at
    1 cycle/row
  - K^T, V projections streamed over Nkv in 512-key chunks
  - S^T = K_h Q_h^T with keys on partitions (row-tiled head pairs, K=64),
    Q^T/K^T kept f32r
  - exp via ACT with fused 1/8 scale + per-key mask bias -> P^T bf16
  - AV in O[q,dh] orientation: stationary P^T q-slices, moving V_aug
    (dh + ones col) bf16, N=65 -> half the moving rows of the O^T form.
    16 accumulators (4 heads x 4 q-tiles) packed 7+7+2 into 3 PSUM banks
    using pending-zero semantics (one start=True per bank).
  - normalize via DVE per-partition reciprocal+scalar-mul, PE transpose
    (identity matmul) to X^T, row-parallel Wo partial, per-q-tile tail
    pipeline.
Host: shards inputs (transposes q/kv once, casts bf16), sums the 4
partials per batch, +bo.

Self-contained: hardcodes all shapes; requires concourse + numpy + ml_dtypes.
"""

import os

import numpy as np
import ml_dtypes

import concourse.bass as bass  # noqa: F401  (bass types via bacc/tile)
import concourse.tile as tile
from concourse import bacc, mybir
from concourse import bass_utils

F32 = mybir.dt.float32
F32R = mybir.dt.float32r
BF16 = mybir.dt.bfloat16
EXP = mybir.ActivationFunctionType.Exp

B, NQ, NKV, D = 2, 512, 4096, 1024
HEADS, DH = 16, 64
SCALE = DH ** -0.5
N_CORES = 8
HPC = HEADS // (N_CORES // B)   # heads per core = 4
PAIRS = HPC // 2                # head pairs per core = 2
KC = 512                        # Nkv chunk size
NCHUNK = NKV // KC              # 8
KO = D // 128                   # 8 contraction sub-chunks
NQT = NQ // 128                 # 4 q tiles

KV_BUFS = int(os.environ.get("KV_BUFS", "2"))
AV_LAG = int(os.environ.get("AV_LAG", "8"))    # units: (c,s,h) entries (4/piece)
PT_BUFS = int(os.environ.get("PT_BUFS", "16"))
PSS_BUFS = int(os.environ.get("PSS_BUFS", "3"))
_NC_CACHE = []


def _build_nc():
    nc = bacc.Bacc("TRN2", target_bir_lowering=False, debug=False,
                   num_devices=N_CORES)
    qT = nc.dram_tensor("qT", [D, NQ], BF16, kind="ExternalInput").ap()
    kvT = nc.dram_tensor("kvT", [D, NKV], BF16, kind="ExternalInput").ap()
    wq = nc.dram_tensor("wq", [D, HPC * DH], BF16, kind="ExternalInput").ap()
    wkvk = nc.dram_tensor("wkvk", [D, HPC * DH], BF16, kind="ExternalInput").ap()
    wkvv = nc.dram_tensor("wkvv", [D, HPC * DH], BF16, kind="ExternalInput").ap()
    wo = nc.dram_tensor("wo", [HPC * DH, D], BF16, kind="ExternalInput").ap()
    bias = nc.dram_tensor("bias", [128, NCHUNK * 4], F32, kind="ExternalInput").ap()
    ident = nc.dram_tensor("ident", [128, 128], BF16, kind="ExternalInput").ap()
    out = nc.dram_tensor("out", [NQ, D], F32, kind="ExternalOutput").ap()

    qT_r = qT.rearrange("(ko p) n -> p ko n", p=128)
    kvT_r = kvT.rearrange("(ko p) n -> p ko n", p=128)
    wq_r = wq.rearrange("(ko p) m -> p ko m", p=128)
    wkvk_r = wkvk.rearrange("(ko p) m -> p ko m", p=128)
    wkvv_r = wkvv.rearrange("(ko p) m -> p ko m", p=128)
    wo_r = wo.rearrange("(ic p) n -> p ic n", p=128)

    with tile.TileContext(nc) as tc:
        with (
            tc.tile_pool(name="const", bufs=1) as cpool,
            tc.tile_pool(name="kv", bufs=KV_BUFS) as kv_pool,
            tc.tile_pool(name="kt", bufs=2) as kt_pool,
            tc.tile_pool(name="v", bufs=2) as v_pool,
            tc.tile_pool(name="pt", bufs=PT_BUFS) as p_pool,
            tc.tile_pool(name="ob", bufs=2) as o_pool,
            tc.tile_pool(name="psA", bufs=1, space="PSUM") as psA,
            tc.tile_pool(name="psV", bufs=1, space="PSUM") as psV,
            tc.tile_pool(name="psS", bufs=PSS_BUFS, space="PSUM") as psS,
            tc.tile_pool(name="psO", bufs=1, space="PSUM") as psO,
        ):
            wq_sb = cpool.tile([128, KO, HPC * DH], BF16, tag="wq")
            wkvk_sb = cpool.tile([128, KO, HPC * DH], BF16, tag="wkvk")
            wkvv_sb = cpool.tile([128, KO, HPC * DH], BF16, tag="wkvv")
            wo_sb = cpool.tile([128, PAIRS, D], BF16, tag="wo")
            qT_sb = cpool.tile([128, KO, NQ], BF16, tag="qT")
            bias_sb = cpool.tile([128, NCHUNK * 4], F32, tag="bias")
            ident_sb = cpool.tile([128, 128], BF16, tag="ident")
            qh_sb = cpool.tile([128, PAIRS, NQ], F32R, tag="qh")
            xT_sb = cpool.tile([128, PAIRS, NQ], BF16, tag="xT")
            xn_sb = [cpool.tile([128, HPC * DH], BF16, tag=f"xn{qt}",
                                name=f"xn{qt}") for qt in range(NQT)]
            rt_sb = [cpool.tile([128, HPC], F32, tag=f"rt{qt}", name=f"rt{qt}")
                     for qt in range(NQT)]

            # first kv chunk + K-proj weights, split per-ko so K-proj can
            # start on strip 0 quickly
            kvc0 = kv_pool.tile([128, KO, KC], BF16, tag="kvc", name="kvc0")
            for ko in range(KO):
                nc.sync.dma_start(wkvk_sb[:, ko, :], wkvk_r[:, ko, :])
                nc.sync.dma_start(kvc0[:, ko, :], kvT_r[:, ko, 0:KC])
            nc.sync.dma_start(wkvv_sb[:], wkvv_r)
            nc.sync.dma_start(wq_sb[:], wq_r)
            nc.sync.dma_start(qT_sb[:], qT_r)
            nc.sync.dma_start(bias_sb[:], bias)
            nc.sync.dma_start(ident_sb[:], ident)

            def q_projection():
                for p in range(PAIRS):
                    qp = psA.tile([128, NQ], F32, tag="psA", name=f"qp{p}")
                    for ko in range(KO):
                        nc.tensor.matmul(
                            qp[:], wq_sb[:, ko, 128 * p:128 * (p + 1)],
                            qT_sb[:, ko, :], start=(ko == 0), stop=(ko == KO - 1),
                        )
                    nc.vector.tensor_copy(qh_sb[:, p, :], qp[:])

            # O accumulators: 16 groups (h, qt) of [128 q, DH+1] f32 packed
            # into 3 PSUM banks (7+7+2). Group g=h*NQT+qt lives at bank
            # g//7, col 65*(g%7). One start=True per bank (slot 0); the
            # pending-zero region mechanism zeroes each group's first write.
            obank = [psO.tile([128, 512], F32, tag=f"ob{b}", name=f"obank{b}")
                     for b in range(3)]

            def o_slice(h, qt, w=DH + 1):
                g = h * NQT + qt
                bk, slot = divmod(g, 7)
                return obank[bk][:, 65 * slot:65 * slot + w], slot

            def load_chunk(c):
                if c == 0:
                    return kvc0
                kvc = kv_pool.tile([128, KO, KC], BF16, tag="kvc", name=f"kvc{c}")
                nc.sync.dma_start(kvc[:], kvT_r[:, :, KC * c:KC * (c + 1)])
                return kvc

            def proj_k_pair(c, kvc, ktc, p):
                kp = psA.tile([128, KC], F32, tag="psA", name=f"kp{c}_{p}")
                for ko in range(KO):
                    nc.tensor.matmul(
                        kp[:], wkvk_sb[:, ko, 128 * p:128 * (p + 1)],
                        kvc[:, ko, :], start=(ko == 0), stop=(ko == KO - 1),
                    )
                nc.vector.tensor_copy(ktc[:, p, :], kp[:])

            def proj_v_sub(c, kvc, vc, m):
                vp = psV.tile([128, HPC * DH], F32, tag="psV", name=f"vp{c}_{m}")
                for ko in range(KO):
                    nc.tensor.matmul(
                        vp[:], kvc[:, ko, 128 * m:128 * (m + 1)],
                        wkvv_sb[:, ko, :], start=(ko == 0), stop=(ko == KO - 1),
                    )
                nc.vector.tensor_copy(
                    vc[:, m, :, 0:DH],
                    vp[:].rearrange("p (h d) -> p h d", h=HPC),
                )

            def alloc_proj_tiles(c):
                ktc = kt_pool.tile([128, PAIRS, KC], F32R, tag="ktc", name=f"ktc{c}")
                vc = v_pool.tile([128, 4, HPC, DH + 1], BF16, tag="vc", name=f"vc{c}")
                nc.vector.memset(vc[:, :, :, DH:DH + 1], 1.0)
                return ktc, vc

            def proj_piece(c, kvc, ktc, vc, s):
                # 4 pieces per chunk, each ~same PE duration as one attention
                # s-piece's QK work
                if s == 0:
                    proj_k_pair(c, kvc, ktc, 0)
                elif s == 1:
                    proj_k_pair(c, kvc, ktc, 1)
                elif s == 2:
                    proj_v_sub(c, kvc, vc, 0)
                    proj_v_sub(c, kvc, vc, 1)
                else:
                    proj_v_sub(c, kvc, vc, 2)
                    proj_v_sub(c, kvc, vc, 3)

            av_pending = []

            def qk_exp_piece(c, ktc, vc, s):
                bias_ap = bias_sb[:, 4 * c + s:4 * c + s + 1]
                for p in range(PAIRS):
                    sps = []
                    for half in range(2):  # row-tiled pair, K=64
                        lo, hi = 64 * half, 64 * (half + 1)
                        sp = psS.tile([128, NQ], F32, tag="psS", name=f"sp{c}_{s}_{p}_{half}")
                        nc.tensor.matmul(
                            sp[:], ktc[lo:hi, p, 128 * s:128 * (s + 1)],
                            qh_sb[lo:hi, p, :], start=True, stop=True,
                        )
                        sps.append(sp)
                    for half, sp in enumerate(sps):
                        h = 2 * p + half
                        pt = p_pool.tile([128, NQ], BF16, tag="pt", name=f"pt{c}_{s}_{p}_{half}")
                        nc.scalar.activation(
                            pt[:], sp[:], EXP, bias=bias_ap, scale=SCALE,
                        )
                        av_pending.append((c, s, h, vc, pt))

            def flush_av(upto):
                # emit AV matmuls for entries that have lagged enough
                while av_pending and len(av_pending) > upto:
                    c, s, h, vc, pt = av_pending.pop(0)
                    for qt in range(NQT):
                        osl, slot = o_slice(h, qt)
                        nc.tensor.matmul(
                            osl, pt[:, 128 * qt:128 * (qt + 1)],
                            vc[:, s, h, :],
                            start=(c == 0 and s == 0 and slot == 0),
                            stop=(c == NCHUNK - 1 and s == 3),
                            skip_group_check=True,
                        )

            # prologue: chunk 0 projections, then Q projection
            kvc_cur = load_chunk(0)
            ktc_cur, vc_cur = alloc_proj_tiles(0)
            for s in range(4):
                proj_piece(0, kvc_cur, ktc_cur, vc_cur, s)
            q_projection()

            # steady state: attention(c) interleaved with projections(c+1);
            # AV lags its exp to hide ACT->PE sem latency
            for c in range(NCHUNK):
                if c + 1 < NCHUNK:
                    kvc_nxt = load_chunk(c + 1)
                    ktc_nxt, vc_nxt = alloc_proj_tiles(c + 1)
                for s in range(4):
                    qk_exp_piece(c, ktc_cur, vc_cur, s)
                    flush_av(AV_LAG)
                    if c + 1 < NCHUNK:
                        proj_piece(c + 1, kvc_nxt, ktc_nxt, vc_nxt, s)
                if c + 1 < NCHUNK:
                    kvc_cur, ktc_cur, vc_cur = kvc_nxt, ktc_nxt, vc_nxt
            flush_av(0)

            nc.sync.dma_start(wo_sb[:], wo_r)

            # tail, pipelined per q-tile:
            #   reciprocal of sums (col 64) -> X = O * (1/s) as bf16 ->
            #   PE transpose to X^T -> row-parallel Wo partial -> out DMA
            wo_pools = [psS, psA, psV]
            for qt in range(NQT):
                for h in range(HPC):
                    osl, _ = o_slice(h, qt)
                    nc.vector.reciprocal(rt_sb[qt][:, h:h + 1], osl[:, 64:65])
                for h in range(HPC):
                    osl, _ = o_slice(h, qt)
                    nc.vector.tensor_scalar_mul(
                        xn_sb[qt][:, DH * h:DH * (h + 1)], osl[:, 0:DH],
                        rt_sb[qt][:, h:h + 1],
                    )
                for ic in range(PAIRS):
                    tp = psV.tile([128, 128], BF16, tag="psV", name=f"tp{qt}_{ic}")
                    nc.tensor.transpose(
                        tp[:], xn_sb[qt][:, 128 * ic:128 * (ic + 1)], ident_sb[:],
                    )
                    nc.scalar.copy(xT_sb[:, ic, 128 * qt:128 * (qt + 1)], tp[:])
                osb = o_pool.tile([128, D], F32, tag="osb", name=f"osb{qt}")
                for n in range(D // 512):
                    pool_w = wo_pools[(qt * (D // 512) + n) % 3]
                    wp = pool_w.tile([128, 512], F32, tag=pool_w.name, name=f"wp{qt}_{n}")
                    for ic in range(PAIRS):
                        nc.tensor.matmul(
                            wp[:], xT_sb[:, ic, 128 * qt:128 * (qt + 1)],
                            wo_sb[:, ic, 512 * n:512 * (n + 1)],
                            start=(ic == 0), stop=(ic == PAIRS - 1),
                        )
                    nc.scalar.copy(osb[:, 512 * n:512 * (n + 1)], wp[:])
                nc.sync.dma_start(out[128 * qt:128 * (qt + 1), :], osb[:])

    nc.compile()
    return nc


def _get_nc():
    if not _NC_CACHE:
        _NC_CACHE.append(_build_nc())
    return _NC_CACHE[0]


LAST_RESULTS = None


def _bf16(x):
    return np.ascontiguousarray(x.astype(ml_dtypes.bfloat16))


def kernel(q, kv, mask, Wq, Wkv, Wo, bo):
    global LAST_RESULTS
    q = np.asarray(q, dtype=np.float32)
    kv = np.asarray(kv, dtype=np.float32)
    mask = np.asarray(mask)
    Wq = np.asarray(Wq, dtype=np.float32)
    Wkv = np.asarray(Wkv, dtype=np.float32)
    Wo = np.asarray(Wo, dtype=np.float32)
    bo = np.asarray(bo, dtype=np.float32)

    inner = HEADS * DH
    qT = [_bf16(q[b].T) for b in range(B)]
    kvT = [_bf16(kv[b].T) for b in range(B)]
    bias = []
    for b in range(B):
        bb = np.where(mask[b], 0.0, -30000.0).astype(np.float32)
        bias.append(np.ascontiguousarray(bb.reshape(NCHUNK * 4, 128).T))
    ident = np.eye(128, dtype=ml_dtypes.bfloat16)

    in_maps = []
    for i in range(N_CORES):
        b, g = divmod(i, N_CORES // B)
        cs = slice(HPC * DH * g, HPC * DH * (g + 1))
        in_maps.append({
            "qT": qT[b],
            "kvT": kvT[b],
            "wq": _bf16(Wq[:, cs]),
            "wkvk": _bf16(Wkv[:, cs]),
            "wkvv": _bf16(Wkv[:, inner:][:, cs]),
            "wo": _bf16(Wo[cs, :]),
            "bias": bias[b],
            "ident": ident,
        })

    nc = _get_nc()
    res = bass_utils.run_bass_kernel_spmd(
        nc, in_maps, core_ids=list(range(N_CORES)))
    LAST_RESULTS = res

    gpb = N_CORES // B
    out = np.zeros((B, NQ, D), np.float32)
    for b in range(B):
        acc = res.results[b * gpb]["out"].astype(np.float32).copy()
        for g in range(1, gpb):
            acc += res.results[b * gpb + g]["out"]
        out[b] = acc + bo[None, :]
    return out


# revision 3
# speedup vs baseline: 1.1548x; 1.0633x over previous
"""Trainium2 Bass kernel for nn_Attention_32762010534254.

Cross-attention: q(B,Nq,D) kv(B,Nkv,D) -> softmax((qWq)(kvWk)^T/sqrt(dh)) (kvWv) Wo + bo
B=2, Nq=512, Nkv=4096, D=1024, heads=16, dh=64.

Sharding (8 cores): core i handles batch b=i//4 and head group g=i%4
(4 heads = 2 pairs). Per-core device work:
  - inputs DMA'd as bf16 (host-side cast); all matmuls stream 1 cycle/row
  - K^T, V projections streamed over Nkv in 512-key chunks; projection
    pieces run ~6 pieces ahead of the attention pieces they feed, so the
    PE never waits on a projection
  - S^T = K_h Q_h^T with keys on partitions (row-tiled head pairs, K=64),
    Q^T/K^T kept f32r
  - exp via ACT with fused 1/8 scale + per-key mask bias -> P^T bf16
  - AV in O[q,dh] orientation: stationary P^T q-slices, moving V_aug
    (dh + ones col) bf16, N=65 -> half the moving rows of the O^T form.
    16 accumulators (4 heads x 4 q-tiles) packed 7+7+2 into 3 PSUM banks
    using pending-zero semantics (one start=True per bank).
    AV work is hoarded mid-stream and drained over the last pieces to
    keep the PE fed where projection work has run out (ACT-bound region).
  - tail: strided per-bank reciprocal of the softmax sums, normalize via
    per-partition-scalar muls split across DVE/ACT, PE transpose
    (identity matmul) to X^T, row-parallel Wo partial, per-(qtile,half)
    out DMA
Host: shards inputs (transposes q/kv once, casts bf16), sums the 4
partials per batch, +bo.

Self-contained: hardcodes all shapes; requires concourse + numpy + ml_dtypes.
"""

import os

import numpy as np
import ml_dtypes

import concourse.bass as bass  # noqa: F401  (bass types via bacc/tile)
import concourse.tile as tile
from concourse import bacc, mybir
from concourse import bass_utils

F32 = mybir.dt.float32
F32R = mybir.dt.float32r
BF16 = mybir.dt.bfloat16
EXP = mybir.ActivationFunctionType.Exp
COPY = mybir.ActivationFunctionType.Copy

B, NQ, NKV, D = 2, 512, 4096, 1024
HEADS, DH = 16, 64
SCALE = DH ** -0.5
N_CORES = 8
HPC = HEADS // (N_CORES // B)   # heads per core = 4
PAIRS = HPC // 2                # head pairs per core = 2
KC = 512                        # Nkv chunk size
NCHUNK = NKV // KC              # 8
KO = D // 128                   # 8 contraction sub-chunks
NQT = NQ // 128                 # 4 q tiles
NPIECE = NCHUNK * 4             # 32 attention/projection pieces

KV_BUFS = int(os.environ.get("KV_BUFS", "3"))
KT_BUFS = int(os.environ.get("KT_BUFS", "3"))
V_BUFS = int(os.environ.get("V_BUFS", "4"))
AV_LAG = int(os.environ.get("AV_LAG", "8"))     # steady-state pending target
AV_HOARD = int(os.environ.get("AV_HOARD", "24"))  # pending before the drain
HOARD_AT = int(os.environ.get("HOARD_AT", "20"))  # piece where hoarding starts
DRAIN_AT = int(os.environ.get("DRAIN_AT", "26"))  # piece where draining starts
PT_BUFS = int(os.environ.get("PT_BUFS", "32"))
PSS_BUFS = int(os.environ.get("PSS_BUFS", "3"))
_NC_CACHE = []


def _build_nc():
    nc = bacc.Bacc("TRN2", target_bir_lowering=False, debug=False,
                   num_devices=N_CORES)
    qT = nc.dram_tensor("qT", [D, NQ], BF16, kind="ExternalInput").ap()
    kvT = nc.dram_tensor("kvT", [D, NKV], BF16, kind="ExternalInput").ap()
    wq = nc.dram_tensor("wq", [D, HPC * DH], BF16, kind="ExternalInput").ap()
    wkvk = nc.dram_tensor("wkvk", [D, HPC * DH], BF16, kind="ExternalInput").ap()
    wkvv = nc.dram_tensor("wkvv", [D, HPC * DH], BF16, kind="ExternalInput").ap()
    wo = nc.dram_tensor("wo", [HPC * DH, D], BF16, kind="ExternalInput").ap()
    bias = nc.dram_tensor("bias", [128, NPIECE], F32, kind="ExternalInput").ap()
    ident = nc.dram_tensor("ident", [128, 128], BF16, kind="ExternalInput").ap()
    out = nc.dram_tensor("out", [NQ, D], F32, kind="ExternalOutput").ap()

    qT_r = qT.rearrange("(ko p) n -> p ko n", p=128)
    kvT_r = kvT.rearrange("(ko p) n -> p ko n", p=128)
    wq_r = wq.rearrange("(ko p) m -> p ko m", p=128)
    wkvk_r = wkvk.rearrange("(ko p) m -> p ko m", p=128)
    wkvv_r = wkvv.rearrange("(ko p) m -> p ko m", p=128)
    wo_r = wo.rearrange("(ic p) n -> p ic n", p=128)

    with tile.TileContext(nc) as tc:
        with (
            tc.tile_pool(name="const", bufs=1) as cpool,
            tc.tile_pool(name="kv", bufs=KV_BUFS) as kv_pool,
            tc.tile_pool(name="kt", bufs=KT_BUFS) as kt_pool,
            tc.tile_pool(name="v", bufs=V_BUFS) as v_pool,
            tc.tile_pool(name="pt", bufs=PT_BUFS) as p_pool,
            tc.tile_pool(name="ob", bufs=4) as o_pool,
            tc.tile_pool(name="psA", bufs=1, space="PSUM") as psA,
            tc.tile_pool(name="psV", bufs=1, space="PSUM") as psV,
            tc.tile_pool(name="psS", bufs=PSS_BUFS, space="PSUM") as psS,
            tc.tile_pool(name="psO", bufs=1, space="PSUM") as psO,
        ):
            wq_sb = cpool.tile([128, KO, HPC * DH], BF16, tag="wq")
            wkvk_sb = cpool.tile([128, KO, HPC * DH], BF16, tag="wkvk")
            wkvv_sb = cpool.tile([128, KO, HPC * DH], BF16, tag="wkvv")
            wo_sb = cpool.tile([128, PAIRS, D], BF16, tag="wo")
            qT_sb = cpool.tile([128, KO, NQ], BF16, tag="qT")
            bias_sb = cpool.tile([128, NPIECE], F32, tag="bias")
            ident_sb = cpool.tile([128, 128], BF16, tag="ident")
            qh_sb = cpool.tile([128, PAIRS, NQ], F32R, tag="qh")
            xT_sb = cpool.tile([128, PAIRS, NQ], BF16, tag="xT")
            xn_sb = [cpool.tile([128, HPC * DH], BF16, tag=f"xn{qt}",
                                name=f"xn{qt}") for qt in range(NQT)]
            rt_sb = [cpool.tile([128, 7], F32, tag=f"rt{b}", name=f"rt{b}")
                     for b in range(3)]

            # prologue DMAs: K weights + chunk0 first so K-proj starts ASAP
            kvc0 = kv_pool.tile([128, KO, KC], BF16, tag="kvc", name="kvc0")
            nc.sync.dma_start(wkvk_sb[:], wkvk_r)
            nc.sync.dma_start(kvc0[:, 0:4, :], kvT_r[:, 0:4, 0:KC])
            nc.sync.dma_start(kvc0[:, 4:8, :], kvT_r[:, 4:8, 0:KC])
            nc.sync.dma_start(wkvv_sb[:], wkvv_r)
            nc.sync.dma_start(wq_sb[:], wq_r)
            nc.sync.dma_start(qT_sb[:], qT_r)
            kvc1 = kv_pool.tile([128, KO, KC], BF16, tag="kvc", name="kvc1")
            nc.sync.dma_start(kvc1[:], kvT_r[:, :, KC:2 * KC])
            nc.sync.dma_start(bias_sb[:], bias)
            nc.sync.dma_start(ident_sb[:], ident)

            def q_projection():
                for p in range(PAIRS):
                    qp = psA.tile([128, NQ], F32, tag="psA", name=f"qp{p}")
                    for ko in range(KO):
                        nc.tensor.matmul(
                            qp[:], wq_sb[:, ko, 128 * p:128 * (p + 1)],
                            qT_sb[:, ko, :], start=(ko == 0), stop=(ko == KO - 1),
                        )
                    nc.vector.tensor_copy(qh_sb[:, p, :], qp[:])

            # O accumulators: 16 groups (h, qt) of [128 q, DH+1] f32 packed
            # into 3 PSUM banks (7+7+2). Group g=h*NQT+qt lives at bank
            # g//7, col 65*(g%7). One start=True per bank (slot 0); the
            # pending-zero region mechanism zeroes each group's first write.
            obank = [psO.tile([128, 512], F32, tag=f"ob{b}", name=f"obank{b}")
                     for b in range(3)]

            def o_slice(h, qt, w=DH + 1):
                g = h * NQT + qt
                bk, slot = divmod(g, 7)
                return obank[bk][:, 65 * slot:65 * slot + w], slot

            kvcs = {0: kvc0, 1: kvc1}

            def prefetch_kvc(c):
                if c in kvcs or c >= NCHUNK:
                    return
                kvc = kv_pool.tile([128, KO, KC], BF16, tag="kvc", name=f"kvc{c}")
                nc.sync.dma_start(kvc[:], kvT_r[:, :, KC * c:KC * (c + 1)])
                kvcs[c] = kvc

            def proj_k_pair(c, ktc, p):
                kvc = kvcs[c]
                kp = psA.tile([128, KC], F32, tag="psA", name=f"kp{c}_{p}")
                for ko in range(KO):
                    nc.tensor.matmul(
                        kp[:], wkvk_sb[:, ko, 128 * p:128 * (p + 1)],
                        kvc[:, ko, :], start=(ko == 0), stop=(ko == KO - 1),
                    )
                nc.vector.tensor_copy(ktc[:, p, :], kp[:])

            def proj_v_sub(c, vc, m):
                kvc = kvcs[c]
                vp = psV.tile([128, HPC * DH], F32, tag="psV", name=f"vp{c}_{m}")
                for ko in range(KO):
                    nc.tensor.matmul(
                        vp[:], kvc[:, ko, 128 * m:128 * (m + 1)],
                        wkvv_sb[:, ko, :], start=(ko == 0), stop=(ko == KO - 1),
                    )
                nc.vector.tensor_copy(
                    vc[:, m, :, 0:DH],
                    vp[:].rearrange("p (h d) -> p h d", h=HPC),
                )

            proj_tiles = {}

            def proj_piece(gp):
                if gp >= NPIECE:
                    return
                c, s = divmod(gp, 4)
                if s == 0:
                    ktc = kt_pool.tile([128, PAIRS, KC], F32R, tag="ktc", name=f"ktc{c}")
                    vc = v_pool.tile([128, 4, HPC, DH + 1], BF16, tag="vc", name=f"vc{c}")
                    nc.vector.memset(vc[:, :, :, DH:DH + 1], 1.0)
                    proj_tiles[c] = (ktc, vc)
                ktc, vc = proj_tiles[c]
                if s == 0:
                    proj_k_pair(c, ktc, 0)
                elif s == 1:
                    proj_k_pair(c, ktc, 1)
                elif s == 2:
                    proj_v_sub(c, vc, 0)
                    proj_v_sub(c, vc, 1)
                else:
                    proj_v_sub(c, vc, 2)
                    proj_v_sub(c, vc, 3)

            av_pending = []

            def qk_exp_piece(c, s):
                ktc, vc = proj_tiles[c]
                bias_ap = bias_sb[:, 4 * c + s:4 * c + s + 1]
                for p in range(PAIRS):
                    sps = []
                    for half in range(2):  # row-tiled pair, K=64
                        lo, hi = 64 * half, 64 * (half + 1)
                        sp = psS.tile([128, NQ], F32, tag="psS", name=f"sp{c}_{s}_{p}_{half}")
                        nc.tensor.matmul(
                            sp[:], ktc[lo:hi, p, 128 * s:128 * (s + 1)],
                            qh_sb[lo:hi, p, :], start=True, stop=True,
                        )
                        sps.append(sp)
                    for half, sp in enumerate(sps):
                        h = 2 * p + half
                        pt = p_pool.tile([128, NQ], BF16, tag="pt", name=f"pt{c}_{s}_{p}_{half}")
                        nc.scalar.activation(
                            pt[:], sp[:], EXP, bias=bias_ap, scale=SCALE,
                        )
                        av_pending.append((c, s, h, vc, pt))

            def flush_av(upto):
                while av_pending and len(av_pending) > upto:
                    c, s, h, vc, pt = av_pending.pop(0)
                    for qt in range(NQT):
                        osl, slot = o_slice(h, qt)
                        nc.tensor.matmul(
                            osl, pt[:, 128 * qt:128 * (qt + 1)],
                            vc[:, s, h, :],
                            start=(c == 0 and s == 0 and slot == 0),
                            stop=(c == NCHUNK - 1 and s == 3),
                            skip_group_check=True,
                        )

            def av_target(a):
                if a < HOARD_AT:
                    return AV_LAG
                if a < DRAIN_AT:
                    return AV_HOARD
                # linear drain to 0 at the last piece
                left = NPIECE - 1 - a
                span = NPIECE - DRAIN_AT
                return (AV_HOARD * left) // span

            # prologue compute: chunk0 projections, Q projection, chunk1 K
            for gp in range(4):
                proj_piece(gp)
            q_projection()
            proj_piece(4)
            proj_piece(5)

            # steady state: attention piece a, projection piece a+6
            for a in range(NPIECE):
                c, s = divmod(a, 4)
                if s == 0:
                    prefetch_kvc(c + 2)
                    if c == 2:
                        nc.sync.dma_start(wo_sb[:], wo_r)
                qk_exp_piece(c, s)
                flush_av(av_target(a))
                proj_piece(a + 6)
            flush_av(0)

            # tail, pipelined per q-tile:
            #   strided per-bank reciprocal of sums (col 64 of each slot) ->
            #   X = O * (1/s) bf16 (DVE/ACT split) -> PE transpose to X^T ->
            #   row-parallel Wo partial -> per-(qt, half) out DMA
            for b, nslot in ((0, 7), (1, 7), (2, 2)):
                sums = obank[b][:, 0:65 * nslot].rearrange(
                    "p (s w) -> p s w", w=65)[:, :, 64:65]
                nc.vector.reciprocal(rt_sb[b][:, 0:nslot], sums)

            wo_pools = [psS, psA, psV]
            tp_pools = [psA, psV]
            for qt in range(NQT):
                for h in range(HPC):
                    osl, _ = o_slice(h, qt)
                    g = h * NQT + qt
                    bk, slot = divmod(g, 7)
                    rt_ap = rt_sb[bk][:, slot:slot + 1]
                    if (qt + h) % 2 == 0:
                        nc.vector.tensor_scalar_mul(
                            xn_sb[qt][:, DH * h:DH * (h + 1)], osl[:, 0:DH], rt_ap)
                    else:
                        nc.scalar.activation(
                            xn_sb[qt][:, DH * h:DH * (h + 1)], osl[:, 0:DH],
                            COPY, scale=rt_ap)
                for ic in range(PAIRS):
                    pool_t = tp_pools[(qt * PAIRS + ic) % 2]
                    tp = pool_t.tile([128, 128], BF16, tag=pool_t.name,
                                     name=f"tp{qt}_{ic}")
                    nc.tensor.transpose(
                        tp[:], xn_sb[qt][:, 128 * ic:128 * (ic + 1)], ident_sb[:],
                    )
                    if ic % 2 == 0:
                        nc.vector.tensor_copy(
                            xT_sb[:, ic, 128 * qt:128 * (qt + 1)], tp[:])
                    else:
                        nc.scalar.copy(
                            xT_sb[:, ic, 128 * qt:128 * (qt + 1)], tp[:])
                for n in range(D // 512):
                    pool_w = wo_pools[(qt * (D // 512) + n) % 3]
                    wp = pool_w.tile([128, 512], F32, tag=pool_w.name, name=f"wp{qt}_{n}")
                    for ic in range(PAIRS):
                        nc.tensor.matmul(
                            wp[:], xT_sb[:, ic, 128 * qt:128 * (qt + 1)],
                            wo_sb[:, ic, 512 * n:512 * (n + 1)],
                            start=(ic == 0), stop=(ic == PAIRS - 1),
                        )
                    osb = o_pool.tile([128, 512], F32, tag="osb", name=f"osb{qt}_{n}")
                    if n % 2 == 0:
                        nc.vector.tensor_copy(osb[:], wp[:])
                    else:
                        nc.scalar.copy(osb[:], wp[:])
                    nc.sync.dma_start(
                        out[128 * qt:128 * (qt + 1), 512 * n:512 * (n + 1)], osb[:])

    nc.compile()
    return nc


def _get_nc():
    if not _NC_CACHE:
        _NC_CACHE.append(_build_nc())
    return _NC_CACHE[0]


LAST_RESULTS = None


def _bf16(x):
    return np.ascontiguousarray(x.astype(ml_dtypes.bfloat16))


def kernel(q, kv, mask, Wq, Wkv, Wo, bo):
    global LAST_RESULTS
    q = np.asarray(q, dtype=np.float32)
    kv = np.asarray(kv, dtype=np.float32)
    mask = np.asarray(mask)
    Wq = np.asarray(Wq, dtype=np.float32)
    Wkv = np.asarray(Wkv, dtype=np.float32)
    Wo = np.asarray(Wo, dtype=np.float32)
    bo = np.asarray(bo, dtype=np.float32)

    inner = HEADS * DH
    qT = [_bf16(q[b].T) for b in range(B)]
    kvT = [_bf16(kv[b].T) for b in range(B)]
    bias = []
    for b in range(B):
        bb = np.where(mask[b], 0.0, -30000.0).astype(np.float32)
        bias.append(np.ascontiguousarray(bb.reshape(NPIECE, 128).T))
    ident = np.eye(128, dtype=ml_dtypes.bfloat16)

    in_maps = []
    for i in range(N_CORES):
        b, g = divmod(i, N_CORES // B)
        cs = slice(HPC * DH * g, HPC * DH * (g + 1))
        in_maps.append({
            "qT": qT[b],
            "kvT": kvT[b],
            "wq": _bf16(Wq[:, cs]),
            "wkvk": _bf16(Wkv[:, cs]),
            "wkvv": _bf16(Wkv[:, inner:][:, cs]),
            "wo": _bf16(Wo[cs, :]),
            "bias": bias[b],
            "ident": ident,
        })

    nc = _get_nc()
    res = bass_utils.run_bass_kernel_spmd(
        nc, in_maps, core_ids=list(range(N_CORES)))
    LAST_RESULTS = res

    gpb = N_CORES // B
    out = np.zeros((B, NQ, D), np.float32)
    for b in range(B):
        acc = res.results[b * gpb]["out"].astype(np.float32).copy()
        for g in range(1, gpb):
            acc += res.results[b * gpb + g]["out"]
        out[b] = acc + bo[None, :]
    return out


# revision 14
# speedup vs baseline: 1.1766x; 1.0189x over previous
"""Trainium2 Bass kernel for nn_Attention_32762010534254.

Cross-attention: q(B,Nq,D) kv(B,Nkv,D) -> softmax((qWq)(kvWk)^T/sqrt(dh)) (kvWv) Wo + bo
B=2, Nq=512, Nkv=4096, D=1024, heads=16, dh=64.

Sharding (8 cores): core i handles batch b=i//4 and head group g=i%4
(4 heads = 2 pairs). Per-core device work:
  - inputs DMA'd as bf16 (host-side cast); all matmuls stream 1 cycle/row
  - K^T, V projections streamed over Nkv in 512-key chunks; projection
    pieces run ~6 pieces ahead of the attention pieces they feed, so the
    PE never waits on a projection
  - S^T = K_h Q_h^T with keys on partitions (row-tiled head pairs, K=64),
    Q^T/K^T kept f32r
  - exp via ACT with fused 1/8 scale + per-key mask bias -> P^T bf16
  - AV in O[q,dh] orientation: stationary P^T q-slices, moving V_aug
    (dh + ones col) bf16, N=65 -> half the moving rows of the O^T form.
    16 accumulators (4 heads x 4 q-tiles) packed 7+7+2 into 3 PSUM banks
    using pending-zero semantics (one start=True per bank).
    AV work is hoarded mid-stream and drained over the last pieces to
    keep the PE fed where projection work has run out (ACT-bound region).
  - tail: strided per-bank reciprocal of the softmax sums, normalize via
    per-partition-scalar muls split across DVE/ACT, PE transpose
    (identity matmul) to X^T, row-parallel Wo partial, per-(qtile,half)
    out DMA
Host: shards inputs (transposes q/kv once, casts bf16), sums the 4
partials per batch, +bo.

Self-contained: hardcodes all shapes; requires concourse + numpy + ml_dtypes.
"""

import os

import numpy as np
import ml_dtypes

import concourse.bass as bass  # noqa: F401  (bass types via bacc/tile)
import concourse.tile as tile
from concourse import bacc, mybir
from concourse import bass_utils

F32 = mybir.dt.float32
F32R = mybir.dt.float32r
BF16 = mybir.dt.bfloat16
EXP = mybir.ActivationFunctionType.Exp
COPY = mybir.ActivationFunctionType.Copy

B, NQ, NKV, D = 2, 512, 4096, 1024
HEADS, DH = 16, 64
SCALE = DH ** -0.5
N_CORES = 8
HPC = HEADS // (N_CORES // B)   # heads per core = 4
PAIRS = HPC // 2                # head pairs per core = 2
KC = 512                        # Nkv chunk size
NCHUNK = NKV // KC              # 8
KO = D // 128                   # 8 contraction sub-chunks
NQT = NQ // 128                 # 4 q tiles
NPIECE = NCHUNK * 4             # 32 attention/projection pieces

KV_BUFS = int(os.environ.get("KV_BUFS", "3"))
KT_BUFS = int(os.environ.get("KT_BUFS", "3"))
V_BUFS = int(os.environ.get("V_BUFS", "6"))
AV_LAG = int(os.environ.get("AV_LAG", "8"))     # steady-state pending target
AV_HOARD = int(os.environ.get("AV_HOARD", "64"))  # pending before the drain
HOARD_AT = int(os.environ.get("HOARD_AT", "10"))  # piece where hoarding starts
DRAIN_AT = int(os.environ.get("DRAIN_AT", "24"))  # piece where draining starts
PT_BUFS = int(os.environ.get("PT_BUFS", "72"))
PSS_BUFS = int(os.environ.get("PSS_BUFS", "3"))
OUT_BF16 = os.environ.get("OUT_BF16", "1") == "1"
_NC_CACHE = []


def _build_nc():
    nc = bacc.Bacc("TRN2", target_bir_lowering=False, debug=False,
                   num_devices=N_CORES)
    qT = nc.dram_tensor("qT", [D, NQ], BF16, kind="ExternalInput").ap()
    kvT = nc.dram_tensor("kvT", [D, NKV], BF16, kind="ExternalInput").ap()
    wq = nc.dram_tensor("wq", [D, HPC * DH], BF16, kind="ExternalInput").ap()
    wkvk = nc.dram_tensor("wkvk", [D, HPC * DH], BF16, kind="ExternalInput").ap()
    wkvv = nc.dram_tensor("wkvv", [D, HPC * DH], BF16, kind="ExternalInput").ap()
    wo = nc.dram_tensor("wo", [HPC * DH, D], BF16, kind="ExternalInput").ap()
    bias = nc.dram_tensor("bias", [128, NPIECE], F32, kind="ExternalInput").ap()
    ident = nc.dram_tensor("ident", [128, 128], BF16, kind="ExternalInput").ap()
    out_dt = BF16 if OUT_BF16 else F32
    out = nc.dram_tensor("out", [NQ, D], out_dt, kind="ExternalOutput").ap()

    qT_r = qT.rearrange("(ko p) n -> p ko n", p=128)
    kvT_r = kvT.rearrange("(ko p) n -> p ko n", p=128)
    wq_r = wq.rearrange("(ko p) m -> p ko m", p=128)
    wkvk_r = wkvk.rearrange("(ko p) m -> p ko m", p=128)
    wkvv_r = wkvv.rearrange("(ko p) m -> p ko m", p=128)
    wo_r = wo.rearrange("(ic p) n -> p ic n", p=128)

    with tile.TileContext(nc) as tc:
        with (
            tc.tile_pool(name="const", bufs=1) as cpool,
            tc.tile_pool(name="kv", bufs=KV_BUFS) as kv_pool,
            tc.tile_pool(name="kt", bufs=KT_BUFS) as kt_pool,
            tc.tile_pool(name="v", bufs=V_BUFS) as v_pool,
            tc.tile_pool(name="pt", bufs=PT_BUFS) as p_pool,
            tc.tile_pool(name="ob", bufs=4) as o_pool,
            tc.tile_pool(name="psA", bufs=1, space="PSUM") as psA,
            tc.tile_pool(name="psV", bufs=1, space="PSUM") as psV,
            tc.tile_pool(name="psS", bufs=PSS_BUFS, space="PSUM") as psS,
            tc.tile_pool(name="psO", bufs=1, space="PSUM") as psO,
        ):
            wq_sb = cpool.tile([128, KO, HPC * DH], BF16, tag="wq")
            wkvk_sb = cpool.tile([128, KO, HPC * DH], BF16, tag="wkvk")
            wkvv_sb = cpool.tile([128, KO, HPC * DH], BF16, tag="wkvv")
            wo_sb = cpool.tile([128, PAIRS, D], BF16, tag="wo")
            qT_sb = cpool.tile([128, KO, NQ], BF16, tag="qT")
            bias_sb = cpool.tile([128, NPIECE], F32, tag="bias")
            ident_sb = cpool.tile([128, 128], BF16, tag="ident")
            qh_sb = cpool.tile([128, PAIRS, NQ], F32R, tag="qh")
            xT_sb = cpool.tile([128, PAIRS, NQ], BF16, tag="xT")
            xn_sb = [cpool.tile([128, HPC * DH], BF16, tag=f"xn{qt}",
                                name=f"xn{qt}") for qt in range(NQT)]
            rt_sb = [cpool.tile([128, 7], F32, tag=f"rt{b}", name=f"rt{b}")
                     for b in range(3)]

            # prologue DMAs: K weights + chunk0 first (split fine so K-proj
            # starts ASAP and streams behind the DMA)
            kvc0 = kv_pool.tile([128, KO, KC], BF16, tag="kvc", name="kvc0")
            nc.sync.dma_start(wkvk_sb[:, 0:4, :], wkvk_r[:, 0:4, :])
            nc.sync.dma_start(kvc0[:, 0:2, :], kvT_r[:, 0:2, 0:KC])
            nc.sync.dma_start(kvc0[:, 2:4, :], kvT_r[:, 2:4, 0:KC])
            nc.sync.dma_start(wkvk_sb[:, 4:8, :], wkvk_r[:, 4:8, :])
            nc.sync.dma_start(kvc0[:, 4:6, :], kvT_r[:, 4:6, 0:KC])
            nc.sync.dma_start(kvc0[:, 6:8, :], kvT_r[:, 6:8, 0:KC])
            nc.sync.dma_start(wkvv_sb[:], wkvv_r)
            nc.sync.dma_start(wq_sb[:], wq_r)
            nc.sync.dma_start(qT_sb[:], qT_r)
            nc.sync.dma_start(bias_sb[:], bias)
            kvc1 = kv_pool.tile([128, KO, KC], BF16, tag="kvc", name="kvc1")
            nc.sync.dma_start(kvc1[:], kvT_r[:, :, KC:2 * KC])

            def q_projection():
                for p in range(PAIRS):
                    qp = psA.tile([128, NQ], F32, tag="psA", name=f"qp{p}")
                    for ko in range(KO):
                        nc.tensor.matmul(
                            qp[:], wq_sb[:, ko, 128 * p:128 * (p + 1)],
                            qT_sb[:, ko, :], start=(ko == 0), stop=(ko == KO - 1),
                        )
                    nc.vector.tensor_copy(qh_sb[:, p, :], qp[:])

            # O accumulators: 16 groups (h, qt) of [128 q, DH+1] f32 packed
            # into 3 PSUM banks (7+7+2). Group g=h*NQT+qt lives at bank
            # g//7, col 65*(g%7). One start=True per bank (slot 0); the
            # pending-zero region mechanism zeroes each group's first write.
            obank = [psO.tile([128, 512], F32, tag=f"ob{b}", name=f"obank{b}")
                     for b in range(3)]

            def o_slice(h, qt, w=DH + 1):
                g = h * NQT + qt
                bk, slot = divmod(g, 7)
                return obank[bk][:, 65 * slot:65 * slot + w], slot

            kvcs = {0: kvc0, 1: kvc1}

            def prefetch_kvc(c):
                if c in kvcs or c >= NCHUNK:
                    return
                kvc = kv_pool.tile([128, KO, KC], BF16, tag="kvc", name=f"kvc{c}")
                nc.sync.dma_start(kvc[:], kvT_r[:, :, KC * c:KC * (c + 1)])
                kvcs[c] = kvc

            def proj_k_pair(c, ktc, p):
                kvc = kvcs[c]
                kp = psA.tile([128, KC], F32, tag="psA", name=f"kp{c}_{p}")
                for ko in range(KO):
                    nc.tensor.matmul(
                        kp[:], wkvk_sb[:, ko, 128 * p:128 * (p + 1)],
                        kvc[:, ko, :], start=(ko == 0), stop=(ko == KO - 1),
                    )
                nc.vector.tensor_copy(ktc[:, p, :], kp[:])

            def proj_v_sub(c, vc, m):
                kvc = kvcs[c]
                vp = psV.tile([128, HPC * DH], F32, tag="psV", name=f"vp{c}_{m}")
                for ko in range(KO):
                    nc.tensor.matmul(
                        vp[:], kvc[:, ko, 128 * m:128 * (m + 1)],
                        wkvv_sb[:, ko, :], start=(ko == 0), stop=(ko == KO - 1),
                    )
                nc.vector.tensor_copy(
                    vc[:, m, :, 0:DH],
                    vp[:].rearrange("p (h d) -> p h d", h=HPC),
                )

            proj_tiles = {}

            def proj_piece(gp):
                if gp >= NPIECE:
                    return
                c, s = divmod(gp, 4)
                if s == 0:
                    ktc = kt_pool.tile([128, PAIRS, KC], F32R, tag="ktc", name=f"ktc{c}")
                    vc = v_pool.tile([128, 4, HPC, DH + 1], BF16, tag="vc", name=f"vc{c}")
                    nc.vector.memset(vc[:, :, :, DH:DH + 1], 1.0)
                    proj_tiles[c] = (ktc, vc)
                ktc, vc = proj_tiles[c]
                if s == 0:
                    proj_k_pair(c, ktc, 0)
                elif s == 1:
                    proj_k_pair(c, ktc, 1)
                elif s == 2:
                    proj_v_sub(c, vc, 0)
                    proj_v_sub(c, vc, 1)
                else:
                    proj_v_sub(c, vc, 2)
                    proj_v_sub(c, vc, 3)

            av_pending = []

            def qk_exp_piece(c, s):
                ktc, vc = proj_tiles[c]
                bias_ap = bias_sb[:, 4 * c + s:4 * c + s + 1]
                for p in range(PAIRS):
                    sps = []
                    for half in range(2):  # row-tiled pair, K=64
                        lo, hi = 64 * half, 64 * (half + 1)
                        sp = psS.tile([128, NQ], F32, tag="psS", name=f"sp{c}_{s}_{p}_{half}")
                        nc.tensor.matmul(
                            sp[:], ktc[lo:hi, p, 128 * s:128 * (s + 1)],
                            qh_sb[lo:hi, p, :], start=True, stop=True,
                        )
                        sps.append(sp)
                    for half, sp in enumerate(sps):
                        h = 2 * p + half
                        pt = p_pool.tile([128, NQ], BF16, tag="pt", name=f"pt{c}_{s}_{p}_{half}")
                        nc.scalar.activation(
                            pt[:], sp[:], EXP, bias=bias_ap, scale=SCALE,
                        )
                        av_pending.append((c, s, h, vc, pt))

            def flush_av(upto):
                while av_pending and len(av_pending) > upto:
                    c, s, h, vc, pt = av_pending.pop(0)
                    for qt in range(NQT):
                        osl, slot = o_slice(h, qt)
                        nc.tensor.matmul(
                            osl, pt[:, 128 * qt:128 * (qt + 1)],
                            vc[:, s, h, :],
                            start=(c == 0 and s == 0 and slot == 0),
                            stop=(c == NCHUNK - 1 and s == 3),
                            skip_group_check=True,
                        )

            def av_target(a):
                if a < HOARD_AT:
                    return AV_LAG
                if a < DRAIN_AT:
                    return AV_HOARD
                # linear drain to 0 at the last piece
                left = NPIECE - 1 - a
                span = NPIECE - DRAIN_AT
                return (AV_HOARD * left) // span

            # prologue compute: chunk0 projections, Q projection, chunk1 K
            for gp in range(4):
                proj_piece(gp)
            q_projection()
            proj_piece(4)
            proj_piece(5)

            # steady state: attention piece a, projection piece a+6
            for a in range(NPIECE):
                c, s = divmod(a, 4)
                if s == 0:
                    prefetch_kvc(c + 2)
                    if c == 2:
                        nc.sync.dma_start(wo_sb[:], wo_r)
                        nc.sync.dma_start(ident_sb[:], ident)
                qk_exp_piece(c, s)
                flush_av(av_target(a))
                proj_piece(a + 6)
            flush_av(0)

            # tail, pipelined per q-tile:
            #   strided per-bank reciprocal of sums (col 64 of each slot) ->
            #   X = O * (1/s) bf16 (DVE/ACT split) -> PE transpose to X^T ->
            #   row-parallel Wo partial -> per-(qt, half) out DMA
            for b, nslot in ((0, 7), (1, 7), (2, 2)):
                sums = obank[b][:, 0:65 * nslot].rearrange(
                    "p (s w) -> p s w", w=65)[:, :, 64:65]
                nc.vector.reciprocal(rt_sb[b][:, 0:nslot], sums)

            wo_pools = [psS, psA, psV]
            tp_pools = [psA, psV]
            # note: GPSIMD cannot access PSUM, so tail work (all psum reads)
            # splits across DVE and ACT only
            mul_engines = [nc.vector, nc.scalar]
            copy_engines = [nc.vector, nc.scalar]
            osb_engines = [nc.scalar, nc.vector]
            for qt in range(NQT):
                for h in range(HPC):
                    osl, _ = o_slice(h, qt)
                    g = h * NQT + qt
                    bk, slot = divmod(g, 7)
                    rt_ap = rt_sb[bk][:, slot:slot + 1]
                    eng = mul_engines[g % 2]
                    if eng is nc.scalar:
                        nc.scalar.activation(
                            xn_sb[qt][:, DH * h:DH * (h + 1)], osl[:, 0:DH],
                            COPY, scale=rt_ap)
                    else:
                        eng.tensor_scalar_mul(
                            xn_sb[qt][:, DH * h:DH * (h + 1)], osl[:, 0:DH], rt_ap)
                for ic in range(PAIRS):
                    j = qt * PAIRS + ic
                    pool_t = tp_pools[j % 2]
                    tp = pool_t.tile([128, 128], BF16, tag=pool_t.name,
                                     name=f"tp{qt}_{ic}")
                    nc.tensor.transpose(
                        tp[:], xn_sb[qt][:, 128 * ic:128 * (ic + 1)], ident_sb[:],
                    )
                    if j % 2 == 0:
                        nc.vector.tensor_copy(
                            xT_sb[:, ic, 128 * qt:128 * (qt + 1)], tp[:])
                    else:
                        nc.scalar.copy(
                            xT_sb[:, ic, 128 * qt:128 * (qt + 1)], tp[:])
                for n in range(D // 512):
                    j = qt * (D // 512) + n
                    pool_w = wo_pools[j % 3]
                    wp = pool_w.tile([128, 512], F32, tag=pool_w.name, name=f"wp{qt}_{n}")
                    for ic in range(PAIRS):
                        nc.tensor.matmul(
                            wp[:], xT_sb[:, ic, 128 * qt:128 * (qt + 1)],
                            wo_sb[:, ic, 512 * n:512 * (n + 1)],
                            start=(ic == 0), stop=(ic == PAIRS - 1),
                        )
                    osb = o_pool.tile([128, 512], out_dt, tag="osb", name=f"osb{qt}_{n}")
                    if j % 2 == 0:
                        nc.scalar.copy(osb[:], wp[:])
                    else:
                        nc.vector.tensor_copy(osb[:], wp[:])
                    nc.sync.dma_start(
                        out[128 * qt:128 * (qt + 1), 512 * n:512 * (n + 1)], osb[:])

    nc.compile()
    return nc


def _get_nc():
    if not _NC_CACHE:
        _NC_CACHE.append(_build_nc())
    return _NC_CACHE[0]


LAST_RESULTS = None


def _bf16(x):
    return np.ascontiguousarray(x.astype(ml_dtypes.bfloat16))


def kernel(q, kv, mask, Wq, Wkv, Wo, bo):
    global LAST_RESULTS
    q = np.asarray(q, dtype=np.float32)
    kv = np.asarray(kv, dtype=np.float32)
    mask = np.asarray(mask)
    Wq = np.asarray(Wq, dtype=np.float32)
    Wkv = np.asarray(Wkv, dtype=np.float32)
    Wo = np.asarray(Wo, dtype=np.float32)
    bo = np.asarray(bo, dtype=np.float32)

    inner = HEADS * DH
    qT = [_bf16(q[b].T) for b in range(B)]
    kvT = [_bf16(kv[b].T) for b in range(B)]
    bias = []
    for b in range(B):
        bb = np.where(mask[b], 0.0, -30000.0).astype(np.float32)
        bias.append(np.ascontiguousarray(bb.reshape(NPIECE, 128).T))
    ident = np.eye(128, dtype=ml_dtypes.bfloat16)

    in_maps = []
    for i in range(N_CORES):
        b, g = divmod(i, N_CORES // B)
        cs = slice(HPC * DH * g, HPC * DH * (g + 1))
        in_maps.append({
            "qT": qT[b],
            "kvT": kvT[b],
            "wq": _bf16(Wq[:, cs]),
            "wkvk": _bf16(Wkv[:, cs]),
            "wkvv": _bf16(Wkv[:, inner:][:, cs]),
            "wo": _bf16(Wo[cs, :]),
            "bias": bias[b],
            "ident": ident,
        })

    nc = _get_nc()
    res = bass_utils.run_bass_kernel_spmd(
        nc, in_maps, core_ids=list(range(N_CORES)))
    LAST_RESULTS = res

    gpb = N_CORES // B
    out = np.zeros((B, NQ, D), np.float32)
    for b in range(B):
        acc = res.results[b * gpb]["out"].astype(np.float32).copy()
        for g in range(1, gpb):
            acc += res.results[b * gpb + g]["out"].astype(np.float32)
        out[b] = acc + bo[None, :]
    return out
